# revision 5
# baseline (speedup 1.0000x reference)
"""LSTM critic kernel for Trainium2 (8 NeuronCores, data-parallel over batch).

Reference computation (per sequence, T=256 steps, hidden H=64):
    gates = [x_t, h] @ W_lstm + b_lstm          # gate order i, j, f, o
    c' = c * sigmoid(f + 1) + sigmoid(i) * tanh(j)
    h' = tanh(c') * sigmoid(o)
    out_t = h' @ W_dec + b_dec

v2 design ("pair" architecture), ACT-throughput-bound:
  - The Activation engine is the hard floor: every sigmoid/tanh element
    costs 0.833ns/col (128 lanes) plus ~185ns fixed per instruction. The
    baseline (3 chains, 6 ACT ops/step) spent 1.1us/step on fixed
    overhead alone. Here the per-core batch of 512 is split into 4
    chains of cb=128, organized as 2 PAIRS; the two chains of a pair run
    in lockstep so one sigmoid op covers both chains' 4 gates
    ([128, 512] psum -> sbuf, 612ns) and one tanh covers both chains' c
    ([64, 256], 398ns): 2 ACT ops per pair-step, ~2.0us/step total.
    The two pairs are staggered ~half a period so the serial recurrence
    latency (~2.6us) is hidden behind ACT throughput.
  - K=105 combined matmul: the xh tile holds [h(64); x(40); 1(bias)] in
    partitions, so each gate group is ONE matmul (vs separate x/h
    matmuls): 4 matmuls [128,128] per pair-step into one psum bank.
  - Weights pre-scaled on host so every gate activation is sigmoid(2*x):
      o' = (o + b_o)/2, i' = (i + b_i)/2, f' = (f + b_f + 1)/2, j' = j + b_j
    tanh(j) = 2*sigmoid(2j) - 1 (tensor_scalar fixup on DVE, 4x mode).
  - On-device decode: out_t = W_dec^T h_t is a transposed matmul per
    chain-step (stationary = h tile [64,128], moving = w_dec [64,1])
    writing column t of a per-chain [128, 256] fp32 psum accumulator.
    This removes ALL per-step output DMA (the baseline streamed 16.8MB
    of h per core) and the host-side decode einsum; output is 512KB of
    y per core, DMA'd once at the end.
  - X input windows: per chain [105, W*cb] tiles, DMA fills partitions
    64:105 (x + bias row), the h-mul writes partitions 0:64 per step.
    One DMA per W=8 steps per chain, triple buffered.
  - DVE cell ops are pair-wide [*, 256] with 2-free-dim APs over the
    sigmoid tile (chain A cols | chain B cols): tj fixup, q = c*sig_f,
    p = tj*sig_i, c' = q + p, then per-chain h-mul into the xh window.
    Partition bases obey walrus' equal-base rule for 2-input DVE ops:
    (o,f) at base 0, (i,j) at base 64, tj parked at base 64.
"""

import os
import sys

for _p in ("/opt/trn_rl_repo", "/root/.axon_site/_ro/trn_rl_repo"):
    if os.path.isdir(_p) and _p not in sys.path:
        sys.path.insert(0, _p)

import numpy as np

from concourse import bass, mybir, tile
from concourse.bass_utils import run_bass_kernel_spmd

# Problem constants (hardcoded per harness contract).
N, T, OBS, ACT, H = 4096, 256, 32, 8, 64
D = OBS + ACT          # 40
DX = D + 1             # x rows incl. the constant-1 bias row
K = H + DX             # 105: [h; x; 1] contraction dim
FORGET_BIAS = 1.0
NCORES = 8
NB = N // NCORES       # 512 sequences per core
CB = 128               # chain batch size
NCH = 4                # chains per core (2 pairs)
NPAIR = 2
XW = 8                 # timesteps per X window DMA

AFT = mybir.ActivationFunctionType
ALU = mybir.AluOpType
BF16 = mybir.dt.bfloat16
F32 = mybir.dt.float32

_BF16_NP = mybir.dt.np(BF16)


def _split_multi_waits(nc, max_waits=1):
    """Workaround for this walrus build's small per-instruction sync-wait
    capacity: hoist excess sem waits onto preceding same-engine NOPs."""
    def stale_first(w):
        nm = (w.ant_name or "")
        return 0 if nm.startswith(("DMA", "PE", "Pool", "SP")) else 1

    for f in nc.m.functions:
        for blk in f.blocks:
            out = []
            changed = False
            for inst in blk.instructions:
                si = inst.sync_info
                if si is not None and si.on_wait and len(si.on_wait) > max_waits:
                    waits = sorted(si.on_wait, key=stale_first)
                    extra, keep = waits[:-max_waits], waits[-max_waits:]
                    for i in range(0, len(extra), max_waits):
                        nop = mybir.InstNoOp(
                            name=f"{inst.name}-wsplit{i}",
                            ins=[],
                            outs=[],
                            engine=inst.engine,
                            sync_info=mybir.SyncInfo(
                                on_wait=extra[i:i + max_waits], on_update=[]
                            ),
                        )
                        out.append(nop)
                    inst.sync_info = mybir.SyncInfo(
                        on_wait=keep, on_update=list(si.on_update)
                    )
                    changed = True
                out.append(inst)
            if changed:
                blk.instructions = out


_ENG_PREFIX = {
    mybir.EngineType.PE: "PE_",
    mybir.EngineType.DVE: "DVE_",
    mybir.EngineType.Activation: "Activation_",
    mybir.EngineType.Pool: "Pool_",
    mybir.EngineType.SP: "SP_",
}


def _drop_same_engine_waits(nc):
    """Remove semaphore waits whose producer ran earlier on the SAME engine
    (engines execute in order, so those waits only add sem-propagation
    stalls)."""
    for f in nc.m.functions:
        for blk in f.blocks:
            for inst in blk.instructions:
                si = inst.sync_info
                if si is None or not si.on_wait:
                    continue
                pref = _ENG_PREFIX.get(inst.engine)
                if pref is None:
                    continue
                keep = [
                    w for w in si.on_wait
                    if not (w.ant_name or "").startswith(pref)
                ]
                if len(keep) != len(si.on_wait):
                    inst.sync_info = mybir.SyncInfo(
                        on_wait=keep, on_update=list(si.on_update)
                    )


def _prep_weights(W_lstm, b_lstm):
    """Build (W1c, W2c): [K=105, 128] combined [h; x; bias] weight blocks.

    Row layout: rows 0:64 = W_h, rows 64:104 = W_x, row 104 = bias.
    Column layout: W1c = [o | i], W2c = [f | j] (64 cols each), scaled so
    sigmoid(2*pre) gives the right gate value; j unscaled (tanh identity).
    """
    W = np.asarray(W_lstm, np.float64)
    b = np.asarray(b_lstm, np.float64)
    W_x, W_h = W[:D], W[D:]
    cols = {k: slice(i * H, (i + 1) * H) for i, k in enumerate("ijfo")}

    def blocks(gate, scale, bias_add):
        wx = W_x[:, cols[gate]] * scale          # [40, 64]
        wh = W_h[:, cols[gate]] * scale          # [64, 64]
        bb = (b[cols[gate]] + bias_add) * scale  # [64]
        return np.concatenate([wh, wx, bb[None, :]], axis=0)  # [105, 64]

    co = blocks("o", 0.5, 0.0)
    ci = blocks("i", 0.5, 0.0)
    cf = blocks("f", 0.5, FORGET_BIAS)
    cj = blocks("j", 1.0, 0.0)
    W1c = np.concatenate([co, ci], axis=1)  # [105, 128] -> parts o:0:64, i:64:128
    W2c = np.concatenate([cf, cj], axis=1)  # [105, 128] -> parts f:0:64, j:64:128
    return W1c, W2c


def _build_nc():
    """Build the SPMD bass program (identical on all 8 cores)."""
    nc = bass.Bass()
    X = nc.declare_dram_parameter("x", [T, DX, NB], BF16, isOutput=False)
    W1d = nc.declare_dram_parameter("w1c", [K, 2 * H], BF16, isOutput=False)
    W2d = nc.declare_dram_parameter("w2c", [K, 2 * H], BF16, isOutput=False)
    WDd = nc.declare_dram_parameter("wdec", [H, 1], BF16, isOutput=False)
    # y output: [NCH, 128, T] fp32 per core
    Y = nc.declare_dram_parameter("y_out", [NCH, CB, T], F32, isOutput=True)

    NW = T // XW  # x windows per chain

    with tile.TileContext(nc) as tc:
        with (
            tc.tile_pool(name="wpool", bufs=1) as wpool,
            tc.tile_pool(name="xh", bufs=3) as xhp,
            tc.tile_pool(name="gps", bufs=2, space="PSUM") as gpsp,
            tc.tile_pool(name="yps", bufs=1, space="PSUM") as ypsp,
            tc.tile_pool(name="sig", bufs=3) as sigp,
            tc.tile_pool(name="cell", bufs=4) as cellp,
        ):
            w1 = wpool.tile([K, 2 * H], BF16, tag="w1")
            w2 = wpool.tile([K, 2 * H], BF16, tag="w2")
            wd = wpool.tile([H, 1], BF16, tag="wd")
            nc.sync.dma_start(w1[:], W1d[:])
            nc.sync.dma_start(w2[:], W2d[:])
            nc.sync.dma_start(wd[:], WDd[:])

            # xh windows: [K, XW*CB] per chain; parts 64:105 DMA'd x, parts
            # 0:64 h (written per step by the h-mul).
            xwin = [{} for _ in range(NCH)]

            def load_xwin(ch, w):
                xt = xhp.tile(
                    [K, XW * CB], BF16, tag=f"xh{ch}", name=f"xh{ch}_{w}"
                )
                nc.sync.dma_start(
                    xt[H:K, :],
                    X[w * XW:(w + 1) * XW, :, ch * CB:(ch + 1) * CB].rearrange(
                        "t f n -> f t n"
                    ),
                )
                xwin[ch][w] = xt

            for ch in range(NCH):
                load_xwin(ch, 0)
                load_xwin(ch, 1)

            # y accumulators: [128, T] fp32 psum per chain.
            ytiles = [
                ypsp.tile([CB, T], F32, tag=f"y{ch}", name=f"y{ch}")
                for ch in range(NCH)
            ]

            # h=0 init: memset parts 0:64 of window-0 col 0 per chain.
            c_cur = [None] * NPAIR
            for ch in range(NCH):
                nc.vector.memset(xwin[ch][0][0:H, 0:CB], 0.0)
            for pr in range(NPAIR):
                c0 = cellp.tile([H, 2 * CB], BF16, tag=f"c{pr}", bufs=2,
                                name=f"c{pr}_init")
                nc.vector.memset(c0[:], 0.0)
                c_cur[pr] = c0

            def hslot(ch, t):
                """AP of [h_slot; x] column block for step t of chain ch."""
                w, col = divmod(t, XW)
                return xwin[ch][w], col

            for t in range(T):
                if t % XW == 0:
                    w = t // XW + 2  # prefetch the window after next
                    if w < NW:
                        for ch in range(NCH):
                            load_xwin(ch, w)

                # Phase-grouped emission per pair; pairs independent.
                pss, ss, cns, tcs = {}, {}, {}, {}
                for pr in range(NPAIR):
                    chA, chB = 2 * pr, 2 * pr + 1
                    ps = gpsp.tile(
                        [2 * H, 4 * CB], F32, tag=f"ps{pr}", name=f"ps{pr}_{t}"
                    )
                    pss[pr] = ps
                    # Column layout: [g1-A | g1-B | g2-A | g2-B] (CB each),
                    # so every gate slice below is a flat contiguous AP.
                    for ci, ch in ((0, chA), (1, chB)):
                        xt, col = hslot(ch, t)
                        rhs = xt[:, col * CB:(col + 1) * CB]
                        nc.tensor.matmul(
                            ps[:, ci * CB:(ci + 1) * CB], w1[:], rhs,
                            start=True, stop=True,
                        )
                        nc.tensor.matmul(
                            ps[:, (2 + ci) * CB:(3 + ci) * CB], w2[:], rhs,
                            start=True, stop=True,
                        )
                for pr in range(NPAIR):
                    # One sigmoid over the whole pair tile.
                    s = sigp.tile(
                        [2 * H, 4 * CB], BF16, tag=f"s{pr}", name=f"s{pr}_{t}"
                    )
                    ss[pr] = s
                    nc.scalar.activation(s[:], pss[pr][:], AFT.Sigmoid,
                                         scale=2.0)
                for pr in range(NPAIR):
                    s = ss[pr]
                    # Cols 0:256 = g1 (A|B), 256:512 = g2 (A|B); parts
                    # 0:64 = (o, f), 64:128 = (i, j). All flat APs.
                    sig_o = s[0:H, 0:2 * CB]
                    sig_i = s[H:2 * H, 0:2 * CB]
                    sig_f = s[0:H, 2 * CB:4 * CB]
                    sig_2j = s[H:2 * H, 2 * CB:4 * CB]

                    tj = cellp.tile(
                        [2 * H, 2 * CB], BF16, tag=f"tj{pr}", name=f"tj{pr}_{t}"
                    )
                    nc.vector.tensor_scalar(
                        tj[H:2 * H, :], sig_2j, 2.0, -1.0, ALU.mult, ALU.add,
                    )
                    q = cellp.tile([H, 2 * CB], BF16, tag=f"q{pr}",
                                   name=f"q{pr}_{t}")
                    nc.vector.tensor_mul(q[:], c_cur[pr][:], sig_f)
                    p = cellp.tile([H, 2 * CB], BF16, tag=f"p{pr}",
                                   name=f"p{pr}_{t}")
                    nc.vector.tensor_mul(p[:], tj[H:2 * H, :], sig_i)
                    c_new = cellp.tile([H, 2 * CB], BF16, tag=f"c{pr}", bufs=2,
                                       name=f"c{pr}_{t}")
                    nc.vector.tensor_add(c_new[:], q[:], p[:])
                    cns[pr] = c_new
                    c_cur[pr] = c_new
                    tc_t = cellp.tile([H, 2 * CB], BF16, tag=f"tc{pr}",
                                      name=f"tc{pr}_{t}")
                    nc.scalar.activation(tc_t[:], c_new[:], AFT.Tanh)
                    tcs[pr] = tc_t
                for pr in range(NPAIR):
                    chA, chB = 2 * pr, 2 * pr + 1
                    s, tc_t = ss[pr], tcs[pr]
                    # h-mul per chain into the next step's xh slot.
                    for ci, ch in ((0, chA), (1, chB)):
                        sig_o_ch = s[0:H, ci * CB:(ci + 1) * CB]
                        if t + 1 < T:
                            xt, col = hslot(ch, t + 1)
                            hdst = xt[0:H, col * CB:(col + 1) * CB]
                        else:
                            hfin = cellp.tile([H, CB], BF16, tag=f"hf{ch}",
                                              bufs=1, name=f"hfin{ch}")
                            hdst = hfin[:]
                        nc.vector.tensor_mul(
                            hdst, tc_t[:, ci * CB:(ci + 1) * CB], sig_o_ch
                        )
                        # decode: y[:, t] = h_t^T @ w_dec
                        nc.tensor.matmul(
                            ytiles[ch][:, t:t + 1], hdst, wd[:],
                            start=True, stop=True,
                        )

            # copy y psum -> sbuf and DMA out
            for ch in range(NCH):
                ysb = sigp.tile([CB, T], F32, tag=f"ysb{ch}", bufs=1,
                                name=f"ysb{ch}")
                nc.scalar.copy(ysb[:], ytiles[ch][:])
                nc.sync.dma_start(Y[ch], ysb[:])

    _drop_same_engine_waits(nc)
    _split_multi_waits(nc)
    return nc


_NC_CACHE = None


def _get_nc():
    global _NC_CACHE
    if _NC_CACHE is None:
        _NC_CACHE = _build_nc()
    return _NC_CACHE


def kernel(obss, actions, W_lstm, b_lstm, W_dec, b_dec, _trace=False):
    obss = np.asarray(obss, np.float32)
    actions = np.asarray(actions, np.float32)

    # Host prep: x = [obs | act | 1] in feature-major per-core layout.
    x = np.concatenate(
        [obss, actions, np.ones((N, T, 1), np.float32)], axis=-1
    )  # [N, T, 41]
    W1c, W2c = _prep_weights(W_lstm, b_lstm)
    wmaps = {
        "w1c": W1c.astype(_BF16_NP),
        "w2c": W2c.astype(_BF16_NP),
        "wdec": np.asarray(W_dec, np.float32).astype(_BF16_NP),
    }

    in_maps = []
    for c in range(NCORES):
        xc = np.ascontiguousarray(
            x[c * NB:(c + 1) * NB].transpose(1, 2, 0)
        ).astype(_BF16_NP)  # [T, 41, NB]
        in_maps.append({"x": xc, **wmaps})

    nc = _get_nc()
    res = run_bass_kernel_spmd(nc, in_maps, list(range(NCORES)), trace=_trace)

    # y shards [NCH, CB, T] -> [T, N]; add decode bias on host.
    ys = np.concatenate(
        [res.results[c]["y_out"].reshape(NB, T) for c in range(NCORES)],
        axis=0,
    )  # [N, T]
    out = ys.T[:, :, None].astype(np.float32) + np.float32(
        np.asarray(b_dec, np.float32)[0]
    )
    if _trace:
        kernel.last_results = res
    return out


# revision 11
# speedup vs baseline: 1.0988x; 1.0988x over previous
"""LSTM critic kernel for Trainium2 (8 NeuronCores, data-parallel over batch).

Reference computation (per sequence, T=256 steps, hidden H=64):
    gates = [x_t, h] @ W_lstm + b_lstm          # gate order i, j, f, o
    c' = c * sigmoid(f + 1) + sigmoid(i) * tanh(j)
    h' = tanh(c') * sigmoid(o)
    out_t = h' @ W_dec + b_dec

The kernel is bound by T * max(L, ACT_busy/step) where L is the serial
per-step chain (h-matmul -> sigmoid -> cell update -> tanh -> h-mul ->
next h-matmul, ~2.4us) and ACT_busy is the Activation engine's per-step
work (3 chains x (sigmoid F=2cb + tanh F=cb + 2x185ns access overhead)
~2.4us). NCH=3, cb~171 sits at the analytic optimum of that tradeoff;
the wins over the previous baseline come from removing every source of
scheduling noise that held the achieved period above that floor:

  - K=105 combined matmul: each xh window tile holds [h(64); x(40); 1]
    in partitions, so each gate group is ONE matmul (engine cost depends
    only on output columns, so folding the x-part into the h-matmul is
    free) -> 6 matmuls/step instead of 12, one psum bank per chain
    instead of two, and the sigmoid reads a flat contiguous [128, 2cb]
    psum AP.
  - The h-mul writes h_t straight into the NEXT step's xh window slot:
    there are no h output windows and no per-step output DMA at all.
  - On-device decode: out_t = W_dec^T h_t as tiny transposed matmuls
    (stationary = h slice, moving = w_dec [64,1], Ldweights is free on
    the PE cost model) accumulating into a per-chain [128, 2T] fp32
    psum tile, DEFERRED one step so the Ldweights' wait on h_t never
    head-of-line-blocks the PE queue (the next gate matmul consumed h_t
    already). Output DMA: 3 x 2KB at the end (vs 16.8MB of h streaming).
  - Weights pre-scaled on host so every gate activation is sigmoid(2*x):
      o' = (o + b_o)/2, i' = (i + b_i)/2, f' = (f + b_f + 1)/2, j' = j + b_j
    tanh(j) = 2*sigmoid(2j) - 1 (one tensor_scalar fixup on DVE, 4x mode).
  - (o, f) gates sit at partition base 0 and (i, j) at base 64 because
    walrus requires equal SBUF base partitions for 2-input DVE ops.
  - The last chain's h-mul is deferred into the next step's emission
    (its tanh lands ~2/3 of a period late; emitting its h-mul in the
    current step's DVE stream would head-of-line block the lead chain).
"""

import os
import sys

for _p in ("/opt/trn_rl_repo", "/root/.axon_site/_ro/trn_rl_repo"):
    if os.path.isdir(_p) and _p not in sys.path:
        sys.path.insert(0, _p)

import numpy as np

from concourse import bass, mybir, tile
from concourse.bass_utils import run_bass_kernel_spmd

# Problem constants (hardcoded per harness contract).
N, T, OBS, ACT, H = 4096, 256, 32, 8, 64
D = OBS + ACT          # 40
DX = D + 1             # x rows incl. the constant-1 bias row
K = H + DX             # 105: [h; x; 1] contraction dim
FORGET_BIAS = 1.0
NCORES = 8
NB = N // NCORES       # 512 sequences per core
SZS = [172, 170, 170]  # chain batch sizes (sum = NB)
NCH = len(SZS)
OFFS = [sum(SZS[:i]) for i in range(NCH + 1)]
XW = 8                 # timesteps per X window DMA

AFT = mybir.ActivationFunctionType
ALU = mybir.AluOpType
BF16 = mybir.dt.bfloat16
F32 = mybir.dt.float32

_BF16_NP = mybir.dt.np(BF16)


def _split_multi_waits(nc, max_waits=1):
    """Workaround for this walrus build's small per-instruction sync-wait
    capacity: hoist excess sem waits onto preceding same-engine NOPs."""
    def stale_first(w):
        nm = (w.ant_name or "")
        return 0 if nm.startswith(("DMA", "PE", "Pool", "SP")) else 1

    for f in nc.m.functions:
        for blk in f.blocks:
            out = []
            changed = False
            for inst in blk.instructions:
                si = inst.sync_info
                if si is not None and si.on_wait and len(si.on_wait) > max_waits:
                    waits = sorted(si.on_wait, key=stale_first)
                    extra, keep = waits[:-max_waits], waits[-max_waits:]
                    for i in range(0, len(extra), max_waits):
                        nop = mybir.InstNoOp(
                            name=f"{inst.name}-wsplit{i}",
                            ins=[],
                            outs=[],
                            engine=inst.engine,
                            sync_info=mybir.SyncInfo(
                                on_wait=extra[i:i + max_waits], on_update=[]
                            ),
                        )
                        out.append(nop)
                    inst.sync_info = mybir.SyncInfo(
                        on_wait=keep, on_update=list(si.on_update)
                    )
                    changed = True
                out.append(inst)
            if changed:
                blk.instructions = out


_ENG_PREFIX = {
    mybir.EngineType.PE: "PE_",
    mybir.EngineType.DVE: "DVE_",
    mybir.EngineType.Activation: "Activation_",
    mybir.EngineType.Pool: "Pool_",
    mybir.EngineType.SP: "SP_",
}


def _drop_same_engine_waits(nc):
    """Remove semaphore waits whose producer ran earlier on the SAME engine
    (in-order execution makes them redundant; they only add sem stalls)."""
    for f in nc.m.functions:
        for blk in f.blocks:
            for inst in blk.instructions:
                si = inst.sync_info
                if si is None or not si.on_wait:
                    continue
                pref = _ENG_PREFIX.get(inst.engine)
                if pref is None:
                    continue
                keep = [
                    w for w in si.on_wait
                    if not (w.ant_name or "").startswith(pref)
                ]
                if len(keep) != len(si.on_wait):
                    inst.sync_info = mybir.SyncInfo(
                        on_wait=keep, on_update=list(si.on_update)
                    )


def _prep_weights(W_lstm, b_lstm):
    """Build (W1c, W2c): [K=105, 128] combined [h; x; bias] weight blocks.

    Row layout: rows 0:64 = W_h, 64:104 = W_x, row 104 = bias.
    W1c cols = [o | i], W2c cols = [f | j]; o/i/f scaled by 0.5 (and f's
    bias gets +FORGET_BIAS) so sigmoid(2*pre) is the gate value; j kept
    unscaled for the tanh(j) = 2*sigmoid(2j)-1 identity.
    """
    W = np.asarray(W_lstm, np.float64)
    b = np.asarray(b_lstm, np.float64)
    W_x, W_h = W[:D], W[D:]
    cols = {k: slice(i * H, (i + 1) * H) for i, k in enumerate("ijfo")}

    def blocks(gate, scale, bias_add):
        wx = W_x[:, cols[gate]] * scale
        wh = W_h[:, cols[gate]] * scale
        bb = (b[cols[gate]] + bias_add) * scale
        return np.concatenate([wh, wx, bb[None, :]], axis=0)  # [105, 64]

    co = blocks("o", 0.5, 0.0)
    ci = blocks("i", 0.5, 0.0)
    cf = blocks("f", 0.5, FORGET_BIAS)
    cj = blocks("j", 1.0, 0.0)
    W1c = np.concatenate([co, ci], axis=1)  # [105,128]: parts 0:64=o, 64:128=i
    W2c = np.concatenate([cf, cj], axis=1)  # [105,128]: parts 0:64=f, 64:128=j
    return W1c, W2c


def _build_nc():
    """Build the SPMD bass program (identical on all 8 cores)."""
    nc = bass.Bass()
    X = nc.declare_dram_parameter("x", [T, DX, NB], BF16, isOutput=False)
    W1d = nc.declare_dram_parameter("w1c", [K, 2 * H], BF16, isOutput=False)
    W2d = nc.declare_dram_parameter("w2c", [K, 2 * H], BF16, isOutput=False)
    WDd = nc.declare_dram_parameter("wdec", [H, 1], BF16, isOutput=False)
    # y output: per chain [128, 2T] fp32; col 2t = seqs 0:128 of the chain,
    # col 2t+1 = seqs 128:cb.
    Y = nc.declare_dram_parameter("y_out", [NCH, 2 * H, 2 * T], F32,
                                  isOutput=True)

    NW = T // XW

    with tile.TileContext(nc) as tc:
        with (
            tc.tile_pool(name="wpool", bufs=1) as wpool,
            tc.tile_pool(name="xh", bufs=3) as xhp,
            tc.tile_pool(name="gps", bufs=1, space="PSUM") as gpsp,
            tc.tile_pool(name="yps", bufs=1, space="PSUM") as ypsp,
            tc.tile_pool(name="sig", bufs=3) as sigp,
            tc.tile_pool(name="small", bufs=6) as smallp,
            tc.tile_pool(name="cst", bufs=4) as cstp,
        ):
            w1 = wpool.tile([K, 2 * H], BF16, tag="w1")
            w2 = wpool.tile([K, 2 * H], BF16, tag="w2")
            wd = wpool.tile([H, 1], BF16, tag="wd")
            nc.sync.dma_start(w1[:], W1d[:])
            nc.sync.dma_start(w2[:], W2d[:])
            nc.sync.dma_start(wd[:], WDd[:])

            # xh windows: [K, XW*cb] per chain; parts 64:105 = x (DMA),
            # parts 0:64 = h (written per step by the h-mul).
            xwin = [{} for _ in range(NCH)]

            def load_xwin(ch, w):
                cb = SZS[ch]
                xt = xhp.tile(
                    [K, XW * cb], BF16, tag=f"xh{ch}", name=f"xh{ch}_{w}"
                )
                nc.sync.dma_start(
                    xt[H:K, :],
                    X[w * XW:(w + 1) * XW, :,
                      OFFS[ch]:OFFS[ch + 1]].rearrange("t f n -> f t n"),
                )
                xwin[ch][w] = xt

            for ch in range(NCH):
                load_xwin(ch, 0)
                load_xwin(ch, 1)

            # y accumulators: [128, 2T] fp32 psum per chain.
            ytiles = []
            for ch in range(NCH):
                yt = ypsp.tile([2 * H, 2 * T], F32, tag=f"y{ch}",
                               name=f"y{ch}")
                ytiles.append(yt)

            # init: h_0 = 0 (col 0 of window 0), c_0 = 0.
            c_cur = [None] * NCH
            for ch in range(NCH):
                cb = SZS[ch]
                nc.vector.memset(xwin[ch][0][0:H, 0:cb], 0.0)
                c0 = cstp.tile([H, cb], BF16, tag=f"c{ch}", name=f"c{ch}_init")
                nc.vector.memset(c0[:], 0.0)
                c_cur[ch] = c0

            def hslot(ch, t):
                """(window tile, col) holding [h_{t-1}; x_t] for step t."""
                w, col = divmod(t, XW)
                return xwin[ch][w], col

            def emit_hmul(ch, t, tc_t, s, prio_bump=15):
                # h_t = tanh(c_t) * sig(o_t), written into step t+1's slot.
                cb = SZS[ch]
                if t + 1 < T:
                    xt, col = hslot(ch, t + 1)
                    hdst = xt[0:H, col * cb:(col + 1) * cb]
                else:
                    hfin = smallp.tile([H, cb], BF16, tag=f"hf{ch}", bufs=1,
                                       name=f"hfin{ch}")
                    hdst = hfin[:]
                bi = nc.vector.tensor_mul(hdst, tc_t[:], s[0:H, 0:cb])
                if prio_bump and getattr(bi.ins, "bass_priority", None) is not None:
                    bi.ins.bass_priority += prio_bump
                return hdst

            # decode matmuls, deferred one step (see module docstring)
            pending_decode = []
            pending_hm = None
            hdsts = {}

            for t in range(T):
                if pending_hm is not None:
                    pch, pt, ptc, ps_ = pending_hm
                    hdsts[pch] = (pt, emit_hmul(pch, pt, ptc, ps_, prio_bump=15))
                    pending_hm = None
                if t % XW == 0:
                    w = t // XW + 2  # prefetch the window after next
                    if w < NW:
                        for ch in range(NCH):
                            load_xwin(ch, w)

                # gate matmuls: one K=105 matmul per gate group per chain.
                pss, ss, tcs = {}, {}, {}
                for ch in range(NCH):
                    cb = SZS[ch]
                    ps = gpsp.tile(
                        [2 * H, 512], F32, tag=f"ps{ch}", name=f"ps{ch}_{t}"
                    )
                    pss[ch] = ps
                    xt, col = hslot(ch, t)
                    rhs = xt[:, col * cb:(col + 1) * cb]
                    nc.tensor.matmul(
                        ps[:, 0:cb], w1[:], rhs, start=True, stop=True,
                    )
                    nc.tensor.matmul(
                        ps[:, cb:2 * cb], w2[:], rhs, start=True, stop=True,
                    )
                # deferred decode for step t-1 (h already consumed above)
                for dch, dt_, hsrc in pending_decode:
                    dcb = SZS[dch]
                    nc.tensor.matmul(
                        ytiles[dch][:, 2 * dt_:2 * dt_ + 1],
                        hsrc[:, 0:2 * H], wd[:], start=True, stop=True,
                    )
                    nc.tensor.matmul(
                        ytiles[dch][0:dcb - 2 * H, 2 * dt_ + 1:2 * dt_ + 2],
                        hsrc[:, 2 * H:dcb], wd[:], start=True, stop=True,
                    )
                pending_decode = []

                for ch in range(NCH):
                    cb = SZS[ch]
                    # one sigmoid over both gate groups: flat [128, 2cb].
                    s = sigp.tile(
                        [2 * H, 2 * cb], BF16, tag=f"s{ch}", name=f"s{ch}_{t}"
                    )
                    ss[ch] = s
                    nc.scalar.activation(s[:], pss[ch][:, 0:2 * cb],
                                         AFT.Sigmoid, scale=2.0)
                for ch in range(NCH):
                    cb = SZS[ch]
                    # s layout: cols 0:cb = g1 (o | i), cb:2cb = g2 (f | j).
                    sig_o = ss[ch][0:H, 0:cb]
                    sig_i = ss[ch][H:2 * H, 0:cb]
                    sig_f = ss[ch][0:H, cb:2 * cb]
                    sig_2j = ss[ch][H:2 * H, cb:2 * cb]

                    tj = smallp.tile(
                        [2 * H, cb], BF16, tag=f"tj{ch}", name=f"tj{ch}_{t}"
                    )
                    nc.vector.tensor_scalar(
                        tj[H:2 * H, :], sig_2j, 2.0, -1.0, ALU.mult, ALU.add,
                    )
                    q = smallp.tile([H, cb], BF16, tag=f"q{ch}",
                                    name=f"q{ch}_{t}")
                    nc.vector.tensor_mul(q[:], c_cur[ch][:], sig_f)
                    p = smallp.tile([H, cb], BF16, tag=f"p{ch}",
                                    name=f"p{ch}_{t}")
                    nc.vector.tensor_mul(p[:], tj[H:2 * H, :], sig_i)
                    c_new = cstp.tile([H, cb], BF16, tag=f"c{ch}",
                                      name=f"c{ch}_{t}")
                    nc.vector.tensor_add(c_new[:], p[:], q[:])
                    c_cur[ch] = c_new
                    tc_t = smallp.tile([H, cb], BF16, tag=f"tc{ch}",
                                       name=f"tc{ch}_{t}")
                    nc.scalar.activation(tc_t[:], c_new[:], AFT.Tanh)
                    tcs[ch] = tc_t
                    # h-muls one chain late (chain ch-1's tanh completes
                    # about when chain ch's cell ops issue).
                    if ch >= 1:
                        hdsts[ch - 1] = (
                            t, emit_hmul(ch - 1, t, tcs[ch - 1], ss[ch - 1])
                        )
                pending_hm = (NCH - 1, t, tcs[NCH - 1], ss[NCH - 1])
                for ch in sorted(hdsts):
                    tdec, hdst = hdsts[ch]
                    pending_decode.append((ch, tdec, hdst))
                hdsts = {}

            # drain: last h-mul + the last two steps' decodes
            pch, pt, ptc, ps_ = pending_hm
            hd = emit_hmul(pch, pt, ptc, ps_)
            pending_decode.append((pch, pt, hd))
            for dch, dt_, hsrc in pending_decode:
                dcb = SZS[dch]
                nc.tensor.matmul(
                    ytiles[dch][:, 2 * dt_:2 * dt_ + 1],
                    hsrc[:, 0:2 * H], wd[:], start=True, stop=True,
                )
                nc.tensor.matmul(
                    ytiles[dch][0:dcb - 2 * H, 2 * dt_ + 1:2 * dt_ + 2],
                    hsrc[:, 2 * H:dcb], wd[:], start=True, stop=True,
                )

            # y psum -> sbuf -> DRAM
            for ch in range(NCH):
                ysb = sigp.tile([2 * H, 2 * T], F32, tag=f"ysb{ch}", bufs=1,
                                name=f"ysb{ch}")
                nc.scalar.copy(ysb[:], ytiles[ch][:])
                nc.sync.dma_start(Y[ch], ysb[:])

    _drop_same_engine_waits(nc)
    _split_multi_waits(nc)
    return nc


_NC_CACHE = None


def _get_nc():
    global _NC_CACHE
    if _NC_CACHE is None:
        _NC_CACHE = _build_nc()
    return _NC_CACHE


def kernel(obss, actions, W_lstm, b_lstm, W_dec, b_dec, _trace=False):
    obss = np.asarray(obss, np.float32)
    actions = np.asarray(actions, np.float32)

    # Host prep: x = [obs | act | 1] in feature-major per-core layout.
    x = np.concatenate(
        [obss, actions, np.ones((N, T, 1), np.float32)], axis=-1
    )  # [N, T, 41]
    W1c, W2c = _prep_weights(W_lstm, b_lstm)
    wmaps = {
        "w1c": W1c.astype(_BF16_NP),
        "w2c": W2c.astype(_BF16_NP),
        "wdec": np.asarray(W_dec, np.float32).astype(_BF16_NP),
    }

    in_maps = []
    for c in range(NCORES):
        xc = np.ascontiguousarray(
            x[c * NB:(c + 1) * NB].transpose(1, 2, 0)
        ).astype(_BF16_NP)  # [T, 41, NB]
        in_maps.append({"x": xc, **wmaps})

    nc = _get_nc()
    res = run_bass_kernel_spmd(nc, in_maps, list(range(NCORES)), trace=_trace)

    # y shards: [NCH, 128, 2T] -> [T, N]; add the decode bias on host.
    out = np.empty((T, N), np.float32)
    tt = np.arange(T)
    for c in range(NCORES):
        yc = res.results[c]["y_out"]  # [NCH, 128, 2T]
        for ch in range(NCH):
            cb = SZS[ch]
            base = c * NB + OFFS[ch]
            out[:, base:base + 2 * H] = yc[ch][:, 2 * tt].T
            out[:, base + 2 * H:base + cb] = yc[ch][0:cb - 2 * H, 2 * tt + 1].T
    out = out[:, :, None] + np.float32(np.asarray(b_dec, np.float32)[0])
    if _trace:
        kernel.last_results = res
    return out


# revision 12
# speedup vs baseline: 1.1067x; 1.0071x over previous
"""LSTM critic kernel for Trainium2 (8 NeuronCores, data-parallel over batch).

Reference computation (per sequence, T=256 steps, hidden H=64):
    gates = [x_t, h] @ W_lstm + b_lstm          # gate order i, j, f, o
    c' = c * sigmoid(f + 1) + sigmoid(i) * tanh(j)
    h' = tanh(c') * sigmoid(o)
    out_t = h' @ W_dec + b_dec

The kernel is latency-bound: the per-step period equals the serial
dependency chain of one chain (h-matmul -> sigmoid -> cell update ->
tanh -> h-mul -> next h-matmul), roughly 2.6us; chains exist only to
keep engines busy inside that latency, and the Activation engine's
per-step busy (one sigmoid + one tanh per chain-step, each with ~185ns
fixed overhead) must stay below the chain latency. NCH=3 balances the
two (ACT ~2.4us busy < ~2.6us chain latency).

Device strategy (per core, batch shard of 512 split into NCH chains):
  - X input (with a constant-1 row carrying the biases) is streamed into
    SBUF in XCHUNK-step windows per chain (one DMA per window, triple
    buffered) — no per-step input DMAs.
  - PSUM tile per chain is PADDED to [128, 1024] fp32 so the two gate
    column groups land in DIFFERENT 2KB psum banks: group1 (o,i) at
    cols [0:CB], group2 (f,j) at cols [512:512+CB]. Accumulation groups
    are per-bank, so BOTH x-part matmuls (K=41, no h dependency) fire
    early, and only the two h-part matmuls (K=64) sit on the critical
    path. One sigmoid ACT op still covers both groups via a 2-free-dim
    access pattern [[512,2],[1,CB]] — same 2*CB element cost.
  - Weights pre-scaled on host so every gate activation is sigmoid(2*x):
      o' = (o + b_o)/2, i' = (i + b_i)/2, f' = (f + b_f + 1)/2, j' = j + b_j
    tanh(j) = 2*sigmoid(2j) - 1 (one cheap tensor_scalar fixup on DVE).
  - One sigmoid ACT op per chain-step over the whole PSUM tile; tanh(c') is
    the only other ACT op (same table set, no reloads). Cell update
    (tj, q, p, c') and the h-mul run on DVE — putting any of them on
    GPSIMD was measured slower end-to-end because Pool ops (~600ns +
    semaphore hops) sit on the c' critical path.
  - h_new is written directly into a rotating [H, OCHUNK*CB] output window
    (double buffered); ONE output DMA per OCHUNK steps per chain on the SP
    HWDGE path. This removes the per-step DMA descriptor-generator load
    (~625-1000ns per DMA) of one-DMA-per-step designs.
  - (o, f) gates sit at partition base 0 and (i, j) at base 64 because
    walrus requires equal SBUF base partitions for 2-input DVE ops.
  - Decode (hs @ W_dec + b_dec) runs on host over the gathered h output.
"""

import os
import sys

for _p in ("/opt/trn_rl_repo", "/root/.axon_site/_ro/trn_rl_repo"):
    if os.path.isdir(_p) and _p not in sys.path:
        sys.path.insert(0, _p)

import numpy as np

from concourse import bass, mybir, tile
from concourse.bass_utils import run_bass_kernel_spmd

# Problem constants (hardcoded per harness contract).
N, T, OBS, ACT, H = 4096, 256, 32, 8, 64
D = OBS + ACT          # 40
DX = D + 1             # x rows incl. the constant-1 bias row
FORGET_BIAS = 1.0
NCORES = 8
NB = N // NCORES       # 512 sequences per core
SZS = [172, 170, 170]  # chain batch sizes (sum = NB)
NCH = len(SZS)
OFFS = [sum(SZS[:i]) for i in range(NCH + 1)]
XCHUNK = 4             # timesteps per X window DMA
OCHUNK = 4             # timesteps per h output window DMA
PSW = 512              # psum column pitch between the two gate groups

AFT = mybir.ActivationFunctionType
ALU = mybir.AluOpType
BF16 = mybir.dt.bfloat16
F32 = mybir.dt.float32

_BF16_NP = mybir.dt.np(BF16)


def _split_multi_waits(nc, max_waits=1):
    """Workaround for this walrus build's small per-instruction sync-wait
    capacity: hoist excess sem waits onto preceding same-engine NOPs.

    Engines execute in order, so a NOP carrying some of the waits right
    before the real instruction preserves semantics exactly.
    """
    def stale_first(w):
        nm = (w.ant_name or "")
        # DMA / PE / Pool sems are usually stale WAR edges; ACT/DVE sems
        # are usually the live RAW producer edge — keep those on the op.
        return 0 if nm.startswith(("DMA", "PE", "Pool", "SP")) else 1

    for f in nc.m.functions:
        for blk in f.blocks:
            out = []
            changed = False
            for inst in blk.instructions:
                si = inst.sync_info
                if si is not None and si.on_wait and len(si.on_wait) > max_waits:
                    waits = sorted(si.on_wait, key=stale_first)
                    extra, keep = waits[:-max_waits], waits[-max_waits:]
                    for i in range(0, len(extra), max_waits):
                        nop = mybir.InstNoOp(
                            name=f"{inst.name}-wsplit{i}",
                            ins=[],
                            outs=[],
                            engine=inst.engine,
                            sync_info=mybir.SyncInfo(
                                on_wait=extra[i:i + max_waits], on_update=[]
                            ),
                        )
                        out.append(nop)
                    inst.sync_info = mybir.SyncInfo(
                        on_wait=keep, on_update=list(si.on_update)
                    )
                    changed = True
                out.append(inst)
            if changed:
                blk.instructions = out


_ENG_PREFIX = {
    mybir.EngineType.PE: "PE_",
    mybir.EngineType.DVE: "DVE_",
    mybir.EngineType.Activation: "Activation_",
    mybir.EngineType.Pool: "Pool_",
    mybir.EngineType.SP: "SP_",
}


def _drop_same_engine_waits(nc):
    """Remove semaphore waits whose producer ran earlier on the SAME engine.

    Engines execute their instruction stream in order, so a wait on a
    semaphore updated by an earlier instruction of the same engine is
    redundant for sequencing (the data hazard is covered by the engine's
    in-order memory pipeline). Tile-framework sem names are prefixed with
    the producer engine, so the instruction's own engine prefix identifies
    droppable waits. This removes the ~100-200ns sem-propagation +
    wait-split-NOP stall between back-to-back dependent ops on one engine.
    """
    for f in nc.m.functions:
        for blk in f.blocks:
            for inst in blk.instructions:
                si = inst.sync_info
                if si is None or not si.on_wait:
                    continue
                pref = _ENG_PREFIX.get(inst.engine)
                if pref is None:
                    continue
                keep = [
                    w for w in si.on_wait
                    if not (w.ant_name or "").startswith(pref)
                ]
                if len(keep) != len(si.on_wait):
                    inst.sync_info = mybir.SyncInfo(
                        on_wait=keep, on_update=list(si.on_update)
                    )


def _prep_weights(W_lstm, b_lstm):
    """Split/scale weights into (W1x, W1h, W2x, W2h).

    Gate pre-activations arranged so sigmoid(2*pre) is the right value:
    i, o, f columns halved (f gets +FORGET_BIAS folded), j kept as-is.
    The x-block rows are [W_x | bias]; the bias rides the constant-1 row.
    """
    W = np.asarray(W_lstm, np.float64)
    b = np.asarray(b_lstm, np.float64)
    W_x, W_h = W[:D], W[D:]
    cols = {k: slice(i * H, (i + 1) * H) for i, k in enumerate("ijfo")}

    def blocks(gate, scale, bias_add):
        wx = W_x[:, cols[gate]] * scale
        wh = W_h[:, cols[gate]] * scale
        bb = (b[cols[gate]] + bias_add) * scale
        return np.concatenate([wx, bb[None, :]], axis=0), wh  # [41,64],[64,64]

    xo, ho = blocks("o", 0.5, 0.0)
    xi, hi = blocks("i", 0.5, 0.0)
    xf, hf = blocks("f", 0.5, FORGET_BIAS)
    xj, hj = blocks("j", 1.0, 0.0)
    # Partition-base pairing: (o, f) at psum parts [0:64], (i, j) at [64:128].
    W1x = np.concatenate([xo, xi], axis=1)  # [41, 128]
    W1h = np.concatenate([ho, hi], axis=1)  # [64, 128]
    W2x = np.concatenate([xf, xj], axis=1)
    W2h = np.concatenate([hf, hj], axis=1)
    return W1x, W1h, W2x, W2h


def _build_nc():
    """Build the SPMD bass program (identical on all 8 cores)."""
    nc = bass.Bass()
    X = nc.declare_dram_parameter("x", [T, DX, NB], BF16, isOutput=False)
    W1xd = nc.declare_dram_parameter("w1x", [DX, 2 * H], BF16, isOutput=False)
    W1hd = nc.declare_dram_parameter("w1h", [H, 2 * H], BF16, isOutput=False)
    W2xd = nc.declare_dram_parameter("w2x", [DX, 2 * H], BF16, isOutput=False)
    W2hd = nc.declare_dram_parameter("w2h", [H, 2 * H], BF16, isOutput=False)
    HS = nc.declare_dram_parameter("hs_out", [T, H, NB], BF16, isOutput=True)

    with tile.TileContext(nc) as tc:
        with (
            tc.tile_pool(name="wpool", bufs=1) as wpool,
            tc.tile_pool(name="xw", bufs=3) as xwp,
            tc.tile_pool(name="how", bufs=2) as howp,
            tc.tile_pool(name="ps", bufs=1, space="PSUM") as psp,
            tc.tile_pool(name="sig", bufs=4) as sigp,
            tc.tile_pool(name="small", bufs=6) as smallp,
            tc.tile_pool(name="cst", bufs=4) as cstp,
        ):
            w1x = wpool.tile([DX, 2 * H], BF16, tag="w1x")
            w1h = wpool.tile([H, 2 * H], BF16, tag="w1h")
            w2x = wpool.tile([DX, 2 * H], BF16, tag="w2x")
            w2h = wpool.tile([H, 2 * H], BF16, tag="w2h")
            nc.sync.dma_start(w1x[:], W1xd[:])
            nc.sync.dma_start(w1h[:], W1hd[:])
            nc.sync.dma_start(w2x[:], W2xd[:])
            nc.sync.dma_start(w2h[:], W2hd[:])

            # X windows: per chain, XCHUNK steps per tile, triple-buffered.
            xwin = [{} for _ in range(NCH)]

            def load_xwin(ch, k):
                cb = SZS[ch]
                csl = slice(OFFS[ch], OFFS[ch + 1])
                xt = xwp.tile(
                    [DX, XCHUNK * cb], BF16, tag=f"xw{ch}", name=f"xw{ch}_{k}"
                )
                nc.sync.dma_start(
                    xt[:],
                    X[k * XCHUNK:(k + 1) * XCHUNK, :, csl].rearrange(
                        "t f n -> f t n"
                    ),
                )
                xwin[ch][k] = xt

            for ch in range(NCH):
                load_xwin(ch, 0)
                load_xwin(ch, 1)

            # h output windows: [H, OCHUNK*cb] per chain, double buffered.
            howin = [None] * NCH

            def new_howin(ch, k):
                cb = SZS[ch]
                howin[ch] = howp.tile(
                    [H, OCHUNK * cb], BF16, tag=f"ho{ch}", name=f"ho{ch}_{k}"
                )

            def store_howin(ch, k):
                # DMA window k (steps k*OCHUNK .. k*OCHUNK+OCHUNK-1) out.
                # Rearrange on the DRAM side so the sbuf AP stays [f, t*n]
                # (contiguous cb-sized runs -> wide descriptors).
                csl = slice(OFFS[ch], OFFS[ch + 1])
                nc.sync.dma_start(
                    HS[k * OCHUNK:(k + 1) * OCHUNK, :, csl].rearrange(
                        "t f n -> f t n"
                    ),
                    howin[ch][:],
                )

            h_cur = [None] * NCH
            c_cur = [None] * NCH
            for ch in range(NCH):
                cb = SZS[ch]
                h0 = smallp.tile([H, cb], BF16, tag=f"h0{ch}", bufs=1,
                                 name=f"h{ch}_init")
                nc.vector.memset(h0[:], 0.0)
                c0 = cstp.tile([H, cb], BF16, tag=f"c{ch}", name=f"c{ch}_init")
                nc.vector.memset(c0[:], 0.0)
                h_cur[ch] = h0
                c_cur[ch] = c0
                new_howin(ch, 0)

            def xslice(ch, t):
                cb = SZS[ch]
                tl = t % XCHUNK
                return xwin[ch][t // XCHUNK][:, tl * cb:(tl + 1) * cb]

            def emit_hmul(ch, t, tc_t, s, prio_bump=15):
                # h = tanh(c') * sig(o), written into the output window slice.
                cb = SZS[ch]
                tl = t % OCHUNK
                h_new = howin[ch][:, tl * cb:(tl + 1) * cb]
                bi = nc.vector.tensor_mul(h_new, tc_t[:], s[0:H, 0:cb])
                if prio_bump and getattr(bi.ins, "bass_priority", None) is not None:
                    # Push this op later in the scheduler's priority order so
                    # it does not head-of-line block the lead chain's cell
                    # update on the DVE (it only becomes ready mid-way through
                    # the next step).
                    bi.ins.bass_priority += prio_bump
                h_cur[ch] = h_new

            # The last chain's h-mul is deferred into the next step's emission:
            # in steady state chain NCH-1's tanh lands ~2/3 of a period late,
            # so emitting its h-mul in the current step's DVE stream would
            # head-of-line block the leading chain's next cell update.
            pending_hm = None

            for t in range(T):
                if pending_hm is not None:
                    emit_hmul(*pending_hm, prio_bump=15)
                    pending_hm = None
                if t % XCHUNK == 0:
                    k = t // XCHUNK + 2  # prefetch the window after next
                    if k < T // XCHUNK:
                        for ch in range(NCH):
                            load_xwin(ch, k)
                if t % OCHUNK == 0 and t > 0:
                    for ch in range(NCH):
                        store_howin(ch, t // OCHUNK - 1)
                        new_howin(ch, t // OCHUNK)

                # Phase-grouped emission across chains: engine sequencers
                # stall in-order on semaphore waits, so a waiting op must not
                # have another chain's ready work queued behind it.
                pss, ss, tjs, qs, ps_, cns, tcs = ({} for _ in range(7))
                # Both x-part matmuls fire early: group1 (cols 0:cb) and
                # group2 (cols PSW:PSW+cb) live in different psum banks, so
                # their accumulation groups are independent; each bank's
                # start/stop pair stays consecutive (x then h).
                for ch in range(NCH):
                    cb = SZS[ch]
                    ps = psp.tile(
                        [2 * H, 2 * PSW], F32, tag=f"ps{ch}", name=f"ps{ch}_{t}"
                    )
                    pss[ch] = ps
                    nc.tensor.matmul(
                        ps[:, PSW:PSW + cb], w2x[:], xslice(ch, t),
                        start=True, stop=False,
                    )
                    nc.tensor.matmul(
                        ps[:, 0:cb], w1x[:], xslice(ch, t),
                        start=True, stop=False,
                    )
                # h-part matmuls: the recurrence head, back-to-back per chain.
                for ch in range(NCH):
                    cb = SZS[ch]
                    nc.tensor.matmul(
                        pss[ch][:, PSW:PSW + cb], w2h[:], h_cur[ch][:],
                        start=False, stop=True,
                    )
                    nc.tensor.matmul(
                        pss[ch][:, 0:cb], w1h[:], h_cur[ch][:],
                        start=False, stop=True,
                    )
                for ch in range(NCH):
                    cb = SZS[ch]
                    # S: parts [0:64] = (sig_o | sig_f), [64:128] = (sig_i | sig_2j)
                    # One ACT op over both psum banks via a 2-free-dim AP.
                    s = sigp.tile(
                        [2 * H, 2 * cb], BF16, tag=f"s{ch}", name=f"s{ch}_{t}"
                    )
                    ss[ch] = s
                    pin = pss[ch][:].rearrange("p (g w) -> p g w", g=2)[:, :, 0:cb]
                    sout = s[:].rearrange("p (g w) -> p g w", g=2)
                    nc.scalar.activation(sout, pin, AFT.Sigmoid, scale=2.0)
                for ch in range(NCH):
                    cb = SZS[ch]
                    # Whole cell update per chain back-to-back on DVE (no
                    # cross-waits inside), so each chain's c' lands as early
                    # as possible for its tanh.
                    tj = smallp.tile(
                        [2 * H, cb], BF16, tag=f"tj{ch}", name=f"tj{ch}_{t}"
                    )
                    tjs[ch] = tj
                    nc.vector.tensor_scalar(
                        tj[H:2 * H, :], ss[ch][H:2 * H, cb:2 * cb],
                        2.0, -1.0, ALU.mult, ALU.add,
                    )
                    q = smallp.tile(
                        [H, cb], BF16, tag=f"q{ch}", name=f"q{ch}_{t}"
                    )
                    qs[ch] = q
                    nc.vector.tensor_mul(
                        q[:], c_cur[ch][:], ss[ch][0:H, cb:2 * cb]
                    )
                    p = smallp.tile(
                        [H, cb], BF16, tag=f"p{ch}", name=f"p{ch}_{t}"
                    )
                    ps_[ch] = p
                    nc.vector.tensor_mul(
                        p[:], tjs[ch][H:2 * H, :], ss[ch][H:2 * H, 0:cb]
                    )
                    c_new = cstp.tile(
                        [H, cb], BF16, tag=f"c{ch}", name=f"c{ch}_{t}"
                    )
                    cns[ch] = c_new
                    nc.vector.tensor_add(c_new[:], ps_[ch][:], qs[ch][:])
                    c_cur[ch] = c_new
                    tc_t = smallp.tile(
                        [H, cb], BF16, tag=f"tc{ch}", name=f"tc{ch}_{t}"
                    )
                    tcs[ch] = tc_t
                    nc.scalar.activation(tc_t[:], cns[ch][:], AFT.Tanh)
                    # Interleave h-muls one chain late so each sits in the
                    # DVE stream at its steady-state ready time (chain ch-1's
                    # tanh completes about when chain ch's cell ops issue).
                    if ch >= 1:
                        emit_hmul(ch - 1, t, tcs[ch - 1], ss[ch - 1])
                pending_hm = (NCH - 1, t, tcs[NCH - 1], ss[NCH - 1])

            emit_hmul(*pending_hm)
            # flush the last output window
            for ch in range(NCH):
                store_howin(ch, T // OCHUNK - 1)

    _drop_same_engine_waits(nc)
    _split_multi_waits(nc)
    return nc


_NC_CACHE = None


def _get_nc():
    global _NC_CACHE
    if _NC_CACHE is None:
        _NC_CACHE = _build_nc()
    return _NC_CACHE


def kernel(obss, actions, W_lstm, b_lstm, W_dec, b_dec, _trace=False):
    obss = np.asarray(obss, np.float32)
    actions = np.asarray(actions, np.float32)

    # Host prep: x = [obs | act | 1] in feature-major per-core layout.
    x = np.concatenate(
        [obss, actions, np.ones((N, T, 1), np.float32)], axis=-1
    )  # [N, T, 41]
    W1x, W1h, W2x, W2h = _prep_weights(W_lstm, b_lstm)
    wmaps = {
        "w1x": W1x.astype(_BF16_NP),
        "w1h": W1h.astype(_BF16_NP),
        "w2x": W2x.astype(_BF16_NP),
        "w2h": W2h.astype(_BF16_NP),
    }

    in_maps = []
    for c in range(NCORES):
        xc = np.ascontiguousarray(
            x[c * NB:(c + 1) * NB].transpose(1, 2, 0)
        ).astype(_BF16_NP)  # [T, 41, NB]
        in_maps.append({"x": xc, **wmaps})

    nc = _get_nc()
    res = run_bass_kernel_spmd(nc, in_maps, list(range(NCORES)), trace=_trace)

    # Gather h shards [T, H, NB] -> [T, H, N]; decode on host.
    hs = np.concatenate(
        [res.results[c]["hs_out"].astype(np.float32) for c in range(NCORES)],
        axis=2,
    )
    wd = np.asarray(W_dec, np.float32)[:, 0]
    out = np.einsum("tfn,f->tn", hs, wd) + np.float32(
        np.asarray(b_dec, np.float32)[0]
    )
    out = out[:, :, None].astype(np.float32)  # [T, N, 1]
    if _trace:
        kernel.last_results = res
    return out



# revision 14
# speedup vs baseline: 1.1084x; 1.0016x over previous
"""LSTM critic kernel for Trainium2 (8 NeuronCores, data-parallel over batch).

Reference computation (per sequence, T=256 steps, hidden H=64):
    gates = [x_t, h] @ W_lstm + b_lstm          # gate order i, j, f, o
    c' = c * sigmoid(f + 1) + sigmoid(i) * tanh(j)
    h' = tanh(c') * sigmoid(o)
    out_t = h' @ W_dec + b_dec

The kernel is latency-bound: the per-step period equals the serial
dependency chain of one chain (h-matmul -> sigmoid -> cell update ->
tanh -> h-mul -> next h-matmul), roughly 2.6us; chains exist only to
keep engines busy inside that latency, and the Activation engine's
per-step busy (one sigmoid + one tanh per chain-step, each with ~185ns
fixed overhead) must stay below the chain latency. NCH=3 balances the
two (ACT ~2.4us busy < ~2.6us chain latency).

Device strategy (per core, batch shard of 512 split into NCH chains):
  - X input (with a constant-1 row carrying the biases) is streamed into
    SBUF in XCHUNK-step windows per chain (one DMA per window, triple
    buffered) — no per-step input DMAs.
  - PSUM tile per chain is PADDED to [128, 1024] fp32 so the two gate
    column groups land in DIFFERENT 2KB psum banks: group1 (o,i) at
    cols [0:CB], group2 (f,j) at cols [512:512+CB]. Accumulation groups
    are per-bank, so BOTH x-part matmuls (K=41, no h dependency) fire
    early, and only the two h-part matmuls (K=64) sit on the critical
    path. One sigmoid ACT op still covers both groups via a 2-free-dim
    access pattern [[512,2],[1,CB]] — same 2*CB element cost.
  - Weights pre-scaled on host so every gate activation is sigmoid(2*x):
      o' = (o + b_o)/2, i' = (i + b_i)/2, f' = (f + b_f + 1)/2, j' = j + b_j
    tanh(j) = 2*sigmoid(2j) - 1 (one cheap tensor_scalar fixup on DVE).
  - One sigmoid ACT op per chain-step over the whole PSUM tile; tanh(c') is
    the only other ACT op (same table set, no reloads). Cell update
    (tj, q, p, c') and the h-mul run on DVE — putting any of them on
    GPSIMD was measured slower end-to-end because Pool ops (~600ns +
    semaphore hops) sit on the c' critical path.
  - h_new is written directly into a rotating [H, OCHUNK*CB] output window
    (double buffered); ONE output DMA per OCHUNK steps per chain on the SP
    HWDGE path. This removes the per-step DMA descriptor-generator load
    (~625-1000ns per DMA) of one-DMA-per-step designs.
  - (o, f) gates sit at partition base 0 and (i, j) at base 64 because
    walrus requires equal SBUF base partitions for 2-input DVE ops.
  - Decode (hs @ W_dec + b_dec) runs on host over the gathered h output.
"""

import os
import sys

for _p in ("/opt/trn_rl_repo", "/root/.axon_site/_ro/trn_rl_repo"):
    if os.path.isdir(_p) and _p not in sys.path:
        sys.path.insert(0, _p)

import numpy as np

from concourse import bass, mybir, tile
from concourse.bass_utils import run_bass_kernel_spmd

# Problem constants (hardcoded per harness contract).
N, T, OBS, ACT, H = 4096, 256, 32, 8, 64
D = OBS + ACT          # 40
DX = D + 1             # x rows incl. the constant-1 bias row
FORGET_BIAS = 1.0
NCORES = 8
NB = N // NCORES       # 512 sequences per core
SZS = [178, 167, 167]  # chain batch sizes (sum = NB)
NCH = len(SZS)
OFFS = [sum(SZS[:i]) for i in range(NCH + 1)]
XCHUNK = 4             # timesteps per X window DMA
OCHUNK = 1             # timesteps per h output window DMA
PSW = 512              # psum column pitch between the two gate groups

AFT = mybir.ActivationFunctionType
ALU = mybir.AluOpType
BF16 = mybir.dt.bfloat16
F32 = mybir.dt.float32

_BF16_NP = mybir.dt.np(BF16)


def _split_multi_waits(nc, max_waits=1):
    """Workaround for this walrus build's small per-instruction sync-wait
    capacity: hoist excess sem waits onto preceding same-engine NOPs.

    Engines execute in order, so a NOP carrying some of the waits right
    before the real instruction preserves semantics exactly.
    """
    def stale_first(w):
        nm = (w.ant_name or "")
        # DMA / PE / Pool sems are usually stale WAR edges; ACT/DVE sems
        # are usually the live RAW producer edge — keep those on the op.
        return 0 if nm.startswith(("DMA", "PE", "Pool", "SP")) else 1

    for f in nc.m.functions:
        for blk in f.blocks:
            out = []
            changed = False
            for inst in blk.instructions:
                si = inst.sync_info
                if si is not None and si.on_wait and len(si.on_wait) > max_waits:
                    waits = sorted(si.on_wait, key=stale_first)
                    extra, keep = waits[:-max_waits], waits[-max_waits:]
                    for i in range(0, len(extra), max_waits):
                        nop = mybir.InstNoOp(
                            name=f"{inst.name}-wsplit{i}",
                            ins=[],
                            outs=[],
                            engine=inst.engine,
                            sync_info=mybir.SyncInfo(
                                on_wait=extra[i:i + max_waits], on_update=[]
                            ),
                        )
                        out.append(nop)
                    inst.sync_info = mybir.SyncInfo(
                        on_wait=keep, on_update=list(si.on_update)
                    )
                    changed = True
                out.append(inst)
            if changed:
                blk.instructions = out


_ENG_PREFIX = {
    mybir.EngineType.PE: "PE_",
    mybir.EngineType.DVE: "DVE_",
    mybir.EngineType.Activation: "Activation_",
    mybir.EngineType.Pool: "Pool_",
    mybir.EngineType.SP: "SP_",
}


def _drop_same_engine_waits(nc):
    """Remove semaphore waits whose producer ran earlier on the SAME engine.

    Engines execute their instruction stream in order, so a wait on a
    semaphore updated by an earlier instruction of the same engine is
    redundant for sequencing (the data hazard is covered by the engine's
    in-order memory pipeline). Tile-framework sem names are prefixed with
    the producer engine, so the instruction's own engine prefix identifies
    droppable waits. This removes the ~100-200ns sem-propagation +
    wait-split-NOP stall between back-to-back dependent ops on one engine.
    """
    for f in nc.m.functions:
        for blk in f.blocks:
            for inst in blk.instructions:
                si = inst.sync_info
                if si is None or not si.on_wait:
                    continue
                pref = _ENG_PREFIX.get(inst.engine)
                if pref is None:
                    continue
                keep = [
                    w for w in si.on_wait
                    if not (w.ant_name or "").startswith(pref)
                ]
                if len(keep) != len(si.on_wait):
                    inst.sync_info = mybir.SyncInfo(
                        on_wait=keep, on_update=list(si.on_update)
                    )


def _prep_weights(W_lstm, b_lstm):
    """Split/scale weights into (W1x, W1h, W2x, W2h).

    Gate pre-activations arranged so sigmoid(2*pre) is the right value:
    i, o, f columns halved (f gets +FORGET_BIAS folded), j kept as-is.
    The x-block rows are [W_x | bias]; the bias rides the constant-1 row.
    """
    W = np.asarray(W_lstm, np.float64)
    b = np.asarray(b_lstm, np.float64)
    W_x, W_h = W[:D], W[D:]
    cols = {k: slice(i * H, (i + 1) * H) for i, k in enumerate("ijfo")}

    def blocks(gate, scale, bias_add):
        wx = W_x[:, cols[gate]] * scale
        wh = W_h[:, cols[gate]] * scale
        bb = (b[cols[gate]] + bias_add) * scale
        return np.concatenate([wx, bb[None, :]], axis=0), wh  # [41,64],[64,64]

    xo, ho = blocks("o", 0.5, 0.0)
    xi, hi = blocks("i", 0.5, 0.0)
    xf, hf = blocks("f", 0.5, FORGET_BIAS)
    xj, hj = blocks("j", 1.0, 0.0)
    # Partition-base pairing: (o, f) at psum parts [0:64], (i, j) at [64:128].
    W1x = np.concatenate([xo, xi], axis=1)  # [41, 128]
    W1h = np.concatenate([ho, hi], axis=1)  # [64, 128]
    W2x = np.concatenate([xf, xj], axis=1)
    W2h = np.concatenate([hf, hj], axis=1)
    return W1x, W1h, W2x, W2h


def _build_nc():
    """Build the SPMD bass program (identical on all 8 cores)."""
    nc = bass.Bass()
    X = nc.declare_dram_parameter("x", [T, DX, NB], BF16, isOutput=False)
    W1xd = nc.declare_dram_parameter("w1x", [DX, 2 * H], BF16, isOutput=False)
    W1hd = nc.declare_dram_parameter("w1h", [H, 2 * H], BF16, isOutput=False)
    W2xd = nc.declare_dram_parameter("w2x", [DX, 2 * H], BF16, isOutput=False)
    W2hd = nc.declare_dram_parameter("w2h", [H, 2 * H], BF16, isOutput=False)
    HS = nc.declare_dram_parameter("hs_out", [T, H, NB], BF16, isOutput=True)

    with tile.TileContext(nc) as tc:
        with (
            tc.tile_pool(name="wpool", bufs=1) as wpool,
            tc.tile_pool(name="xw", bufs=3) as xwp,
            tc.tile_pool(name="how", bufs=2) as howp,
            tc.tile_pool(name="ps", bufs=1, space="PSUM") as psp,
            tc.tile_pool(name="sig", bufs=4) as sigp,
            tc.tile_pool(name="small", bufs=6) as smallp,
            tc.tile_pool(name="cst", bufs=4) as cstp,
        ):
            w1x = wpool.tile([DX, 2 * H], BF16, tag="w1x")
            w1h = wpool.tile([H, 2 * H], BF16, tag="w1h")
            w2x = wpool.tile([DX, 2 * H], BF16, tag="w2x")
            w2h = wpool.tile([H, 2 * H], BF16, tag="w2h")
            nc.sync.dma_start(w1x[:], W1xd[:])
            nc.sync.dma_start(w1h[:], W1hd[:])
            nc.sync.dma_start(w2x[:], W2xd[:])
            nc.sync.dma_start(w2h[:], W2hd[:])

            # X windows: per chain, XCHUNK steps per tile, triple-buffered.
            xwin = [{} for _ in range(NCH)]

            def load_xwin(ch, k):
                cb = SZS[ch]
                csl = slice(OFFS[ch], OFFS[ch + 1])
                xt = xwp.tile(
                    [DX, XCHUNK * cb], BF16, tag=f"xw{ch}", name=f"xw{ch}_{k}"
                )
                nc.sync.dma_start(
                    xt[:],
                    X[k * XCHUNK:(k + 1) * XCHUNK, :, csl].rearrange(
                        "t f n -> f t n"
                    ),
                )
                xwin[ch][k] = xt

            for ch in range(NCH):
                load_xwin(ch, 0)
                load_xwin(ch, 1)

            # h output windows: [H, OCHUNK*cb] per chain, double buffered.
            howin = [None] * NCH

            def new_howin(ch, k):
                cb = SZS[ch]
                howin[ch] = howp.tile(
                    [H, OCHUNK * cb], BF16, tag=f"ho{ch}", name=f"ho{ch}_{k}"
                )

            def store_howin(ch, k):
                # DMA window k (steps k*OCHUNK .. k*OCHUNK+OCHUNK-1) out.
                # Rearrange on the DRAM side so the sbuf AP stays [f, t*n]
                # (contiguous cb-sized runs -> wide descriptors).
                csl = slice(OFFS[ch], OFFS[ch + 1])
                nc.sync.dma_start(
                    HS[k * OCHUNK:(k + 1) * OCHUNK, :, csl].rearrange(
                        "t f n -> f t n"
                    ),
                    howin[ch][:],
                )

            h_cur = [None] * NCH
            c_cur = [None] * NCH
            for ch in range(NCH):
                cb = SZS[ch]
                h0 = smallp.tile([H, cb], BF16, tag=f"h0{ch}", bufs=1,
                                 name=f"h{ch}_init")
                nc.vector.memset(h0[:], 0.0)
                c0 = cstp.tile([H, cb], BF16, tag=f"c{ch}", name=f"c{ch}_init")
                nc.vector.memset(c0[:], 0.0)
                h_cur[ch] = h0
                c_cur[ch] = c0
                new_howin(ch, 0)

            def xslice(ch, t):
                cb = SZS[ch]
                tl = t % XCHUNK
                return xwin[ch][t // XCHUNK][:, tl * cb:(tl + 1) * cb]

            def emit_hmul(ch, t, tc_t, s, prio_bump=15):
                # h = tanh(c') * sig(o), written into the output window slice.
                cb = SZS[ch]
                tl = t % OCHUNK
                h_new = howin[ch][:, tl * cb:(tl + 1) * cb]
                bi = nc.vector.tensor_mul(h_new, tc_t[:], s[0:H, 0:cb])
                if prio_bump and getattr(bi.ins, "bass_priority", None) is not None:
                    # Push this op later in the scheduler's priority order so
                    # it does not head-of-line block the lead chain's cell
                    # update on the DVE (it only becomes ready mid-way through
                    # the next step).
                    bi.ins.bass_priority += prio_bump
                h_cur[ch] = h_new

            # The last chain's h-mul is deferred into the next step's emission:
            # in steady state chain NCH-1's tanh lands ~2/3 of a period late,
            # so emitting its h-mul in the current step's DVE stream would
            # head-of-line block the leading chain's next cell update.
            pending_hm = None

            for t in range(T):
                if pending_hm is not None:
                    emit_hmul(*pending_hm, prio_bump=15)
                    pending_hm = None
                if t % XCHUNK == 0:
                    k = t // XCHUNK + 2  # prefetch the window after next
                    if k < T // XCHUNK:
                        for ch in range(NCH):
                            load_xwin(ch, k)
                if t % OCHUNK == 0 and t > 0:
                    for ch in range(NCH):
                        store_howin(ch, t // OCHUNK - 1)
                        new_howin(ch, t // OCHUNK)

                # Phase-grouped emission across chains: engine sequencers
                # stall in-order on semaphore waits, so a waiting op must not
                # have another chain's ready work queued behind it.
                pss, ss, tjs, qs, ps_, cns, tcs = ({} for _ in range(7))
                # Both x-part matmuls fire early: group1 (cols 0:cb) and
                # group2 (cols PSW:PSW+cb) live in different psum banks, so
                # their accumulation groups are independent; each bank's
                # start/stop pair stays consecutive (x then h).
                for ch in range(NCH):
                    cb = SZS[ch]
                    ps = psp.tile(
                        [2 * H, 2 * PSW], F32, tag=f"ps{ch}", name=f"ps{ch}_{t}"
                    )
                    pss[ch] = ps
                    nc.tensor.matmul(
                        ps[:, PSW:PSW + cb], w2x[:], xslice(ch, t),
                        start=True, stop=False,
                    )
                    nc.tensor.matmul(
                        ps[:, 0:cb], w1x[:], xslice(ch, t),
                        start=True, stop=False,
                    )
                # h-part matmuls: the recurrence head, back-to-back per chain.
                for ch in range(NCH):
                    cb = SZS[ch]
                    nc.tensor.matmul(
                        pss[ch][:, PSW:PSW + cb], w2h[:], h_cur[ch][:],
                        start=False, stop=True,
                    )
                    nc.tensor.matmul(
                        pss[ch][:, 0:cb], w1h[:], h_cur[ch][:],
                        start=False, stop=True,
                    )
                for ch in range(NCH):
                    cb = SZS[ch]
                    # S: parts [0:64] = (sig_o | sig_f), [64:128] = (sig_i | sig_2j)
                    # One ACT op over both psum banks via a 2-free-dim AP.
                    s = sigp.tile(
                        [2 * H, 2 * cb], BF16, tag=f"s{ch}", name=f"s{ch}_{t}"
                    )
                    ss[ch] = s
                    pin = pss[ch][:].rearrange("p (g w) -> p g w", g=2)[:, :, 0:cb]
                    sout = s[:].rearrange("p (g w) -> p g w", g=2)
                    nc.scalar.activation(sout, pin, AFT.Sigmoid, scale=2.0)
                for ch in range(NCH):
                    cb = SZS[ch]
                    # Whole cell update per chain back-to-back on DVE (no
                    # cross-waits inside), so each chain's c' lands as early
                    # as possible for its tanh.
                    tj = smallp.tile(
                        [2 * H, cb], BF16, tag=f"tj{ch}", name=f"tj{ch}_{t}"
                    )
                    tjs[ch] = tj
                    nc.vector.tensor_scalar(
                        tj[H:2 * H, :], ss[ch][H:2 * H, cb:2 * cb],
                        2.0, -1.0, ALU.mult, ALU.add,
                    )
                    q = smallp.tile(
                        [H, cb], BF16, tag=f"q{ch}", name=f"q{ch}_{t}"
                    )
                    qs[ch] = q
                    nc.vector.tensor_mul(
                        q[:], c_cur[ch][:], ss[ch][0:H, cb:2 * cb]
                    )
                    p = smallp.tile(
                        [H, cb], BF16, tag=f"p{ch}", name=f"p{ch}_{t}"
                    )
                    ps_[ch] = p
                    nc.vector.tensor_mul(
                        p[:], tjs[ch][H:2 * H, :], ss[ch][H:2 * H, 0:cb]
                    )
                    c_new = cstp.tile(
                        [H, cb], BF16, tag=f"c{ch}", name=f"c{ch}_{t}"
                    )
                    cns[ch] = c_new
                    nc.vector.tensor_add(c_new[:], ps_[ch][:], qs[ch][:])
                    c_cur[ch] = c_new
                    tc_t = smallp.tile(
                        [H, cb], BF16, tag=f"tc{ch}", name=f"tc{ch}_{t}"
                    )
                    tcs[ch] = tc_t
                    nc.scalar.activation(tc_t[:], cns[ch][:], AFT.Tanh)
                    # Interleave h-muls one chain late so each sits in the
                    # DVE stream at its steady-state ready time (chain ch-1's
                    # tanh completes about when chain ch's cell ops issue).
                    if ch >= 1:
                        emit_hmul(ch - 1, t, tcs[ch - 1], ss[ch - 1])
                pending_hm = (NCH - 1, t, tcs[NCH - 1], ss[NCH - 1])

            emit_hmul(*pending_hm)
            # flush the last output window
            for ch in range(NCH):
                store_howin(ch, T // OCHUNK - 1)

    _drop_same_engine_waits(nc)
    _split_multi_waits(nc)
    return nc


_NC_CACHE = None


def _get_nc():
    global _NC_CACHE
    if _NC_CACHE is None:
        _NC_CACHE = _build_nc()
    return _NC_CACHE


def kernel(obss, actions, W_lstm, b_lstm, W_dec, b_dec, _trace=False):
    obss = np.asarray(obss, np.float32)
    actions = np.asarray(actions, np.float32)

    # Host prep: x = [obs | act | 1] in feature-major per-core layout.
    x = np.concatenate(
        [obss, actions, np.ones((N, T, 1), np.float32)], axis=-1
    )  # [N, T, 41]
    W1x, W1h, W2x, W2h = _prep_weights(W_lstm, b_lstm)
    wmaps = {
        "w1x": W1x.astype(_BF16_NP),
        "w1h": W1h.astype(_BF16_NP),
        "w2x": W2x.astype(_BF16_NP),
        "w2h": W2h.astype(_BF16_NP),
    }

    in_maps = []
    for c in range(NCORES):
        xc = np.ascontiguousarray(
            x[c * NB:(c + 1) * NB].transpose(1, 2, 0)
        ).astype(_BF16_NP)  # [T, 41, NB]
        in_maps.append({"x": xc, **wmaps})

    nc = _get_nc()
    res = run_bass_kernel_spmd(nc, in_maps, list(range(NCORES)), trace=_trace)

    # Gather h shards [T, H, NB] -> [T, H, N]; decode on host.
    hs = np.concatenate(
        [res.results[c]["hs_out"].astype(np.float32) for c in range(NCORES)],
        axis=2,
    )
    wd = np.asarray(W_dec, np.float32)[:, 0]
    out = np.einsum("tfn,f->tn", hs, wd) + np.float32(
        np.asarray(b_dec, np.float32)[0]
    )
    out = out[:, :, None].astype(np.float32)  # [T, N, 1]
    if _trace:
        kernel.last_results = res
    return out

